# revision 21
# baseline (speedup 1.0000x reference)
"""CharRNN Trainium2 kernel, v3: single tanh ACT per step.

State layout: one merged buffer `gob` [128, tc*2B] per chunk; step slot s
occupies cols [s*2B, (s+1)*2B): first B cols = g1-side (rows 0:72 =
h^T[128:200], rows 72:128 = junk), last B cols = g0-side (rows 0:128 =
h^T[0:128]).  The per-step PSUM tile is one bank [128, 2B] written by 6
matmuls (2 early one-hot matmuls vs zero-padded EW tiles + 4 W_h matmuls),
then ONE tanh ACT writes the whole [128, 2B] slot.  Junk rows multiply
zero-padded weight rows, so they never contaminate results.
"""

import sys

sys.path.insert(0, "/opt/trn_rl_repo")

import numpy as np
import ml_dtypes

VOCAB = 33
EMBED = 200
HIDDEN = 200
BATCH = 256
SEQ = 1024
NCORES = 8
BPC = BATCH // NCORES
H0 = 128
H1 = HIDDEN - H0               # 72
KP = 128
MP = 256
OHK = 64                       # one-hot contraction rows (33 used + pad)

_PROG_CACHE = {}


def _build_program(T, nhalf, tc):
    import concourse.mybir as mybir
    from concourse import bacc, tile
    from concourse.masks import make_identity

    f32 = mybir.dt.float32
    bf16 = mybir.dt.bfloat16
    AF = mybir.ActivationFunctionType

    BH = BPC // nhalf
    assert T % tc == 0
    nchunk = T // tc

    nc = bacc.Bacc(None, target_bir_lowering=False)

    ohh_d = [
        nc.dram_tensor(f"ohh{h}", [OHK, (T + 1) * BH], bf16, kind="ExternalInput")
        for h in range(nhalf)
    ]
    gi_d = [
        nc.dram_tensor(f"gi{h}", [KP, 2 * BH], bf16, kind="ExternalInput")
        for h in range(nhalf)
    ]
    wh0_d = nc.dram_tensor("wh0", [KP, MP], bf16, kind="ExternalInput")
    wh1_d = nc.dram_tensor("wh1", [KP, MP], bf16, kind="ExternalInput")
    wo0_d = nc.dram_tensor("wo0", [H0, VOCAB], bf16, kind="ExternalInput")
    wo1_d = nc.dram_tensor("wo1", [KP, VOCAB], bf16, kind="ExternalInput")
    embT_d = nc.dram_tensor("embT", [EMBED, VOCAB], bf16, kind="ExternalInput")
    we_d = nc.dram_tensor("we", [EMBED, HIDDEN], bf16, kind="ExternalInput")
    logits_d = nc.dram_tensor("logits", [BPC, T * VOCAB], f32, kind="ExternalOutput")
    hout_d = nc.dram_tensor("hout", [BPC, HIDDEN], f32, kind="ExternalOutput")

    logits_v = logits_d[:].rearrange("b (t v) -> b t v", v=VOCAB)

    with tile.TileContext(nc) as tcx:
        with (
            tcx.tile_pool(name="const", bufs=1) as constp,
            tcx.tile_pool(name="tmp", bufs=1) as tmpp,
            tcx.tile_pool(name="gop", bufs=3) as gop,
            tcx.tile_pool(name="ohcp", bufs=3) as ohcp,
            tcx.tile_pool(name="lbp", bufs=6) as lbp,
            tcx.tile_pool(name="psp", bufs=5, space="PSUM") as psp,
            tcx.tile_pool(name="pslgp", bufs=3, space="PSUM") as pslgp,
        ):
            # EW = embedding @ W_e -> rows 0:33 of ewp [OHK, MP], rest zero.
            # Its input DMAs go first (longest setup pole); its SBUF->SBUF
            # result writes go on the GpSimd DMA queue so the Sync FIFO
            # never blocks behind the EW matmul pipeline.
            ewp_t = constp.tile([OHK, MP], bf16, tag="ewp")
            nc.vector.memset(ewp_t[:], 0.0)
            embT0 = tmpp.tile([H0, VOCAB], bf16, tag="embT0")
            nc.sync.dma_start(embT0[:], embT_d[0:H0, :])
            embT1 = tmpp.tile([EMBED - H0, VOCAB], bf16, tag="embT1")
            nc.sync.dma_start(embT1[:], embT_d[H0:EMBED, :])
            we0 = tmpp.tile([H0, HIDDEN], bf16, tag="we0")
            nc.sync.dma_start(we0[:], we_d[0:H0, :])
            we1 = tmpp.tile([EMBED - H0, HIDDEN], bf16, tag="we1")
            nc.sync.dma_start(we1[:], we_d[H0:EMBED, :])

            # warm the ACT tanh table early so the ~2.7us table load
            # overlaps the setup DMAs instead of the first real step
            scratch_t = constp.tile([1, 1], f32, tag="scratch")
            nc.scalar.activation(scratch_t[:], scratch_t[:], AF.Tanh)

            wh0_t = constp.tile([KP, MP], bf16, tag="wh0")
            nc.sync.dma_start(wh0_t[:], wh0_d[:])
            wh1_t = constp.tile([KP, MP], bf16, tag="wh1")
            nc.sync.dma_start(wh1_t[:], wh1_d[:])

            psew = pslgp.tile([VOCAB, HIDDEN], f32, tag="pslg")
            nc.tensor.matmul(psew[:], embT0[:], we0[:], start=True, stop=False)
            nc.tensor.matmul(psew[:], embT1[:], we1[:], start=False, stop=True)
            ewsb = tmpp.tile([VOCAB, HIDDEN], bf16, tag="ewsb")
            nc.vector.tensor_copy(ewsb[:], psew[:])
            # EW columns: M0 part (h 0:128) -> ewp[:, 0:128], M1 -> [:, 128:200+pad]
            nc.gpsimd.dma_start(ewp_t[0:VOCAB, 0:H0], ewsb[:, 0:H0])
            nc.gpsimd.dma_start(ewp_t[0:VOCAB, H0:H0 + H1], ewsb[:, H0:HIDDEN])

            # initial state slot, same layout as a gob slot
            g_init = []
            oh_prolog = []
            for h in range(nhalf):
                gi = constp.tile([KP, 2 * BH], bf16, tag=f"ginit{h}")
                nc.sync.dma_start(gi[:], gi_d[h][:])
                g_init.append(gi)
                op0 = constp.tile([OHK, BH], bf16, tag=f"ohprolog{h}")
                nc.sync.dma_start(op0[:], ohh_d[h][:, 0:BH])
                oh_prolog.append(op0)

            # non-critical constants (first used by chunk-1 logits / epilogue):
            # emitted after the step-0-critical DMAs so they don't delay it
            wo0_t = constp.tile([H0, VOCAB], bf16, tag="wo0")
            nc.gpsimd.dma_start(wo0_t[:], wo0_d[:])
            wo1_t = constp.tile([KP, VOCAB], bf16, tag="wo1")
            nc.gpsimd.dma_start(wo1_t[:], wo1_d[:])
            ident_t = constp.tile([128, 128], bf16, tag="ident")
            make_identity(nc, ident_t[:])

            lg_count = [0]

            def emit_logits(cprev, hh, bl, gob, t0=0, ntok=None):
                if ntok is None:
                    ntok = tc
                gv = gob[:].rearrange("p (t c b) -> p t c b", c=2, b=BH)
                g0v = gv[:, t0:t0 + ntok, 1, bl]   # [128, ntok]
                g1v = gv[:, t0:t0 + ntok, 0, bl]   # rows 72:128 junk
                pl = pslgp.tile([ntok, VOCAB], f32, tag="pslg")
                nc.tensor.matmul(pl[:], g0v, wo0_t[:], start=True, stop=False)
                nc.tensor.matmul(pl[:], g1v, wo1_t[:], start=False, stop=True)
                lb = lbp.tile([ntok, VOCAB], f32, tag="lb")
                nc.vector.tensor_copy(lb[:], pl[:])
                bglob = hh * BH + bl
                # alternate DMA queues so back-to-back logits stores overlap
                eng = nc.sync if lg_count[0] % 2 == 0 else nc.gpsimd
                lg_count[0] += 1
                base = cprev * tc + t0
                eng.dma_start(logits_v[bglob, base: base + ntok, :], lb[:])

            prev = [None] * nhalf
            for c in range(nchunk):
                bufs = []
                for h in range(nhalf):
                    gob = gop.tile([KP, tc * 2 * BH], bf16, tag=f"gob{h}")
                    ohc = ohcp.tile([OHK, tc * BH], bf16, tag=f"ohc{h}")
                    base = (c * tc + 1) * BH
                    if c == 0:
                        # chunk 0 is not prefetch-hidden: split the one-hot
                        # load so early steps wait only on a small piece
                        edges = [0, 8, 32, 64, tc]
                        for a, b in zip(edges, edges[1:]):
                            nc.sync.dma_start(
                                ohc[:, a * BH: b * BH],
                                ohh_d[h][:, base + a * BH: base + b * BH],
                            )
                    else:
                        nc.sync.dma_start(
                            ohc[:], ohh_d[h][:, base: base + tc * BH]
                        )
                    bufs.append((gob, ohc))

                n_lg = nhalf * BH
                lg_every = max(1, tc // n_lg)

                for s in range(tc):
                    for h in range(nhalf):
                        gob, ohc = bufs[h]
                        if c == 0 and s == 0:
                            gprev = g_init[h][:]
                            ohcur = oh_prolog[h][:]
                        elif s == 0:
                            gprev = prev[h][0][:, (tc - 1) * 2 * BH: tc * 2 * BH]
                            ohcur = prev[h][1][:, (tc - 1) * BH: tc * BH]
                        else:
                            gprev = gob[:, (s - 1) * 2 * BH: s * 2 * BH]
                            ohcur = ohc[:, (s - 1) * BH: s * BH]
                        g1prev = gprev[:, 0:BH]
                        g0prev = gprev[:, BH:2 * BH]

                        ps = psp.tile([KP, 2 * BH], f32, tag="ps")
                        nc.tensor.matmul(
                            ps[:, 0:BH], ewp_t[:, H0:MP], ohcur,
                            start=True, stop=False,
                        )
                        nc.tensor.matmul(
                            ps[:, BH:2 * BH], ewp_t[:, 0:H0], ohcur,
                            start=False, stop=False, skip_group_check=True,
                        )
                        nc.tensor.matmul(
                            ps[:, 0:BH], wh1_t[:, H0:MP], g1prev,
                            start=False, stop=False, skip_group_check=True,
                        )
                        nc.tensor.matmul(
                            ps[:, 0:BH], wh0_t[:, H0:MP], g0prev,
                            start=False, stop=True, skip_group_check=True,
                        )
                        nc.tensor.matmul(
                            ps[:, BH:2 * BH], wh1_t[:, 0:H0], g1prev,
                            start=False, stop=False, skip_group_check=True,
                        )
                        nc.tensor.matmul(
                            ps[:, BH:2 * BH], wh0_t[:, 0:H0], g0prev,
                            start=False, stop=True, skip_group_check=True,
                        )
                        nc.scalar.activation(
                            gob[:, s * 2 * BH:(s + 1) * 2 * BH], ps[:], AF.Tanh
                        )

                    if c >= 1 and s % lg_every == 0:
                        idx = s // lg_every
                        if idx < n_lg:
                            hh = idx // BH
                            bl = idx % BH
                            emit_logits(c - 1, hh, bl, prev[hh][0])
                prev = bufs

            for idx in range(nhalf * BH):
                hh = idx // BH
                bl = idx % BH
                emit_logits(nchunk - 1, hh, bl, prev[hh][0])

            for h in range(nhalf):
                pg = prev[h][0]
                last = (tc - 1) * 2 * BH
                psh = pslgp.tile([BH, HIDDEN], bf16, tag="pslg")
                nc.tensor.transpose(
                    psh[:, 0:H0], pg[:, last + BH: last + 2 * BH], ident_t[:]
                )
                nc.tensor.transpose(
                    psh[:, H0:HIDDEN],
                    pg[0:H1, last: last + BH],
                    ident_t[0:H1, 0:H1],
                )
                hb = tmpp.tile([BH, HIDDEN], f32, tag=f"hb{h}")
                nc.vector.tensor_copy(hb[:], psh[:])
                nc.gpsimd.dma_start(hout_d[h * BH:(h + 1) * BH, :], hb[:])

    nc.compile()
    return nc


def get_program(T=SEQ, nhalf=1, tc=128):
    key = (T, nhalf, tc)
    if key not in _PROG_CACHE:
        _PROG_CACHE[key] = _build_program(T, nhalf, tc)
    return _PROG_CACHE[key]


def _prep_inputs(x, hidden, embedding, W_e, W_h, W_o, T, nhalf):
    bf16 = ml_dtypes.bfloat16
    BH = BPC // nhalf
    x = np.asarray(x)
    hidden = np.asarray(hidden, dtype=np.float32)
    embedding = np.asarray(embedding, dtype=np.float32)
    W_e = np.asarray(W_e, dtype=np.float32)
    W_h = np.asarray(W_h, dtype=np.float32)
    W_o = np.asarray(W_o, dtype=np.float32)

    wh0 = np.zeros((KP, MP), np.float32)
    wh0[0:H0, 128:128 + H1] = W_h[0:H0, H0:HIDDEN]
    wh0[0:H0, 0:H0] = W_h[0:H0, 0:H0]
    wh1 = np.zeros((KP, MP), np.float32)
    wh1[0:H1, 128:128 + H1] = W_h[H0:HIDDEN, H0:HIDDEN]
    wh1[0:H1, 0:H0] = W_h[H0:HIDDEN, 0:H0]
    wo1 = np.zeros((KP, VOCAB), np.float32)
    wo1[0:H1, :] = W_o[H0:HIDDEN, :]
    shared = {
        "wh0": wh0.astype(bf16),
        "wh1": wh1.astype(bf16),
        "wo0": W_o[0:H0, :].astype(bf16),
        "wo1": wo1.astype(bf16),
        "embT": np.ascontiguousarray(embedding.T).astype(bf16),
        "we": W_e.astype(bf16),
    }
    in_maps = []
    for core in range(NCORES):
        m = dict(shared)
        xc = x[core * BPC:(core + 1) * BPC, :]
        hc = hidden[core * BPC:(core + 1) * BPC, :]
        for h in range(nhalf):
            xh = np.asarray(xc[h * BH:(h + 1) * BH, :T], dtype=np.int64)
            ohh = np.zeros((OHK, (T + 1) * BH), np.float32)
            cols = np.arange(T * BH)
            ohh[xh.T.reshape(-1), cols] = 1.0
            m[f"ohh{h}"] = ohh.astype(bf16)
            gh = np.ascontiguousarray(hc[h * BH:(h + 1) * BH, :].T)  # [200, BH]
            gi = np.zeros((KP, 2 * BH), np.float32)
            gi[0:H1, 0:BH] = gh[H0:HIDDEN, :]      # g1-side
            gi[0:H0, BH:2 * BH] = gh[0:H0, :]      # g0-side
            m[f"gi{h}"] = gi.astype(bf16)
        in_maps.append(m)
    return in_maps


def run_on_device(x, hidden, embedding, W_e, W_h, W_o, T=SEQ, nhalf=1, tc=128,
                  trace=False, **kw):
    from concourse.bass_utils import run_bass_kernel_spmd

    tc = min(tc, T)
    nc = get_program(T, nhalf, tc)
    in_maps = _prep_inputs(x, hidden, embedding, W_e, W_h, W_o, T, nhalf)
    res = run_bass_kernel_spmd(
        nc, in_maps, core_ids=list(range(NCORES)), trace=trace, **kw
    )
    logits = np.concatenate(
        [np.asarray(r["logits"], dtype=np.float32).reshape(BPC, T, VOCAB)
         for r in res.results],
        axis=0,
    )
    hout = np.concatenate(
        [np.asarray(r["hout"], dtype=np.float32) for r in res.results], axis=0
    )
    return (logits, hout), res


def kernel(x, hidden, embedding, W_e, W_h, W_o):
    (logits, hout), _ = run_on_device(x, hidden, embedding, W_e, W_h, W_o)
    return logits, hout


# revision 23
# speedup vs baseline: 1.0103x; 1.0103x over previous
"""CharRNN Trainium2 kernel, v3: single tanh ACT per step.

State layout: one merged buffer `gob` [128, tc*2B] per chunk; step slot s
occupies cols [s*2B, (s+1)*2B): first B cols = g1-side (rows 0:72 =
h^T[128:200], rows 72:128 = junk), last B cols = g0-side (rows 0:128 =
h^T[0:128]).  The per-step PSUM tile is one bank [128, 2B] written by 6
matmuls (2 early one-hot matmuls vs zero-padded EW tiles + 4 W_h matmuls),
then ONE tanh ACT writes the whole [128, 2B] slot.  Junk rows multiply
zero-padded weight rows, so they never contaminate results.
"""

import sys

sys.path.insert(0, "/opt/trn_rl_repo")

import numpy as np
import ml_dtypes

VOCAB = 33
EMBED = 200
HIDDEN = 200
BATCH = 256
SEQ = 1024
NCORES = 8
BPC = BATCH // NCORES
H0 = 128
H1 = HIDDEN - H0               # 72
KP = 128
MP = 256
OHK = 64                       # one-hot contraction rows (33 used + pad)

_PROG_CACHE = {}


def _build_program(T, nhalf, tc):
    import concourse.mybir as mybir
    from concourse import bacc, tile
    from concourse.masks import make_identity

    f32 = mybir.dt.float32
    bf16 = mybir.dt.bfloat16
    AF = mybir.ActivationFunctionType

    BH = BPC // nhalf
    assert T % tc == 0
    nchunk = T // tc

    nc = bacc.Bacc(None, target_bir_lowering=False)

    ohh_d = [
        nc.dram_tensor(f"ohh{h}", [OHK, (T + 1) * BH], bf16, kind="ExternalInput")
        for h in range(nhalf)
    ]
    gi_d = [
        nc.dram_tensor(f"gi{h}", [KP, 2 * BH], bf16, kind="ExternalInput")
        for h in range(nhalf)
    ]
    wh0_d = nc.dram_tensor("wh0", [KP, MP], bf16, kind="ExternalInput")
    wh1_d = nc.dram_tensor("wh1", [KP, MP], bf16, kind="ExternalInput")
    wo0_d = nc.dram_tensor("wo0", [H0, VOCAB], bf16, kind="ExternalInput")
    wo1_d = nc.dram_tensor("wo1", [KP, VOCAB], bf16, kind="ExternalInput")
    embT_d = nc.dram_tensor("embT", [EMBED, VOCAB], bf16, kind="ExternalInput")
    we_d = nc.dram_tensor("we", [EMBED, HIDDEN], bf16, kind="ExternalInput")
    logits_d = nc.dram_tensor("logits", [BPC, T * VOCAB], f32, kind="ExternalOutput")
    hout_d = nc.dram_tensor("hout", [BPC, HIDDEN], f32, kind="ExternalOutput")

    logits_v = logits_d[:].rearrange("b (t v) -> b t v", v=VOCAB)

    with tile.TileContext(nc) as tcx:
        with (
            tcx.tile_pool(name="const", bufs=1) as constp,
            tcx.tile_pool(name="tmp", bufs=1) as tmpp,
            tcx.tile_pool(name="gop", bufs=3) as gop,
            tcx.tile_pool(name="ohcp", bufs=3) as ohcp,
            tcx.tile_pool(name="lbp", bufs=6) as lbp,
            tcx.tile_pool(name="psp", bufs=5, space="PSUM") as psp,
            tcx.tile_pool(name="pslgp", bufs=3, space="PSUM") as pslgp,
        ):
            # EW = embedding @ W_e -> rows 0:33 of ewp [OHK, MP], rest zero.
            # Its input DMAs go first (longest setup pole); its SBUF->SBUF
            # result writes go on the GpSimd DMA queue so the Sync FIFO
            # never blocks behind the EW matmul pipeline.
            ewp_t = constp.tile([OHK, MP], bf16, tag="ewp")
            nc.vector.memset(ewp_t[:], 0.0)
            embT0 = tmpp.tile([H0, VOCAB], bf16, tag="embT0")
            nc.gpsimd.dma_start(embT0[:], embT_d[0:H0, :])
            embT1 = tmpp.tile([EMBED - H0, VOCAB], bf16, tag="embT1")
            nc.gpsimd.dma_start(embT1[:], embT_d[H0:EMBED, :])
            we0 = tmpp.tile([H0, HIDDEN], bf16, tag="we0")
            nc.sync.dma_start(we0[:], we_d[0:H0, :])
            we1 = tmpp.tile([EMBED - H0, HIDDEN], bf16, tag="we1")
            nc.sync.dma_start(we1[:], we_d[H0:EMBED, :])

            # warm the ACT tanh table early so the ~2.7us table load
            # overlaps the setup DMAs instead of the first real step
            scratch_t = constp.tile([1, 1], f32, tag="scratch")
            nc.scalar.activation(scratch_t[:], scratch_t[:], AF.Tanh)

            wh0_t = constp.tile([KP, MP], bf16, tag="wh0")
            nc.sync.dma_start(wh0_t[:], wh0_d[:])
            wh1_t = constp.tile([KP, MP], bf16, tag="wh1")
            nc.sync.dma_start(wh1_t[:], wh1_d[:])

            psew = pslgp.tile([VOCAB, HIDDEN], f32, tag="pslg")
            nc.tensor.matmul(psew[:], embT0[:], we0[:], start=True, stop=False)
            nc.tensor.matmul(psew[:], embT1[:], we1[:], start=False, stop=True)
            ewsb = tmpp.tile([VOCAB, HIDDEN], bf16, tag="ewsb")
            nc.vector.tensor_copy(ewsb[:], psew[:])
            # EW columns: M0 part (h 0:128) -> ewp[:, 0:128], M1 -> [:, 128:200+pad]
            nc.gpsimd.dma_start(ewp_t[0:VOCAB, 0:H0], ewsb[:, 0:H0])
            nc.gpsimd.dma_start(ewp_t[0:VOCAB, H0:H0 + H1], ewsb[:, H0:HIDDEN])

            # initial state slot, same layout as a gob slot
            g_init = []
            oh_prolog = []
            for h in range(nhalf):
                gi = constp.tile([KP, 2 * BH], bf16, tag=f"ginit{h}")
                nc.sync.dma_start(gi[:], gi_d[h][:])
                g_init.append(gi)
                op0 = constp.tile([OHK, BH], bf16, tag=f"ohprolog{h}")
                nc.sync.dma_start(op0[:], ohh_d[h][:, 0:BH])
                oh_prolog.append(op0)

            # non-critical constants (first used by chunk-1 logits / epilogue):
            # emitted after the step-0-critical DMAs so they don't delay it
            wo0_t = constp.tile([H0, VOCAB], bf16, tag="wo0")
            nc.gpsimd.dma_start(wo0_t[:], wo0_d[:])
            wo1_t = constp.tile([KP, VOCAB], bf16, tag="wo1")
            nc.gpsimd.dma_start(wo1_t[:], wo1_d[:])
            ident_t = constp.tile([128, 128], bf16, tag="ident")
            make_identity(nc, ident_t[:])

            lg_count = [0]

            def emit_logits(cprev, hh, bl, gob, t0=0, ntok=None):
                if ntok is None:
                    ntok = tc
                gv = gob[:].rearrange("p (t c b) -> p t c b", c=2, b=BH)
                g0v = gv[:, t0:t0 + ntok, 1, bl]   # [128, ntok]
                g1v = gv[:, t0:t0 + ntok, 0, bl]   # rows 72:128 junk
                pl = pslgp.tile([ntok, VOCAB], f32, tag="pslg")
                nc.tensor.matmul(pl[:], g0v, wo0_t[:], start=True, stop=False)
                nc.tensor.matmul(pl[:], g1v, wo1_t[:], start=False, stop=True)
                lb = lbp.tile([ntok, VOCAB], f32, tag="lb")
                nc.vector.tensor_copy(lb[:], pl[:])
                bglob = hh * BH + bl
                # alternate DMA queues so back-to-back logits stores overlap
                eng = nc.sync if lg_count[0] % 2 == 0 else nc.gpsimd
                lg_count[0] += 1
                base = cprev * tc + t0
                eng.dma_start(logits_v[bglob, base: base + ntok, :], lb[:])

            prev = [None] * nhalf
            for c in range(nchunk):
                bufs = []
                for h in range(nhalf):
                    gob = gop.tile([KP, tc * 2 * BH], bf16, tag=f"gob{h}")
                    ohc = ohcp.tile([OHK, tc * BH], bf16, tag=f"ohc{h}")
                    base = (c * tc + 1) * BH
                    if c == 0:
                        # chunk 0 is not prefetch-hidden: split the one-hot
                        # load so early steps wait only on a small piece
                        edges = [0, 8, 32, 64, tc]
                        for a, b in zip(edges, edges[1:]):
                            nc.sync.dma_start(
                                ohc[:, a * BH: b * BH],
                                ohh_d[h][:, base + a * BH: base + b * BH],
                            )
                    else:
                        nc.sync.dma_start(
                            ohc[:], ohh_d[h][:, base: base + tc * BH]
                        )
                    bufs.append((gob, ohc))

                n_lg = nhalf * BH
                lg_every = max(1, tc // n_lg)

                for s in range(tc):
                    for h in range(nhalf):
                        gob, ohc = bufs[h]
                        if c == 0 and s == 0:
                            gprev = g_init[h][:]
                            ohcur = oh_prolog[h][:]
                        elif s == 0:
                            gprev = prev[h][0][:, (tc - 1) * 2 * BH: tc * 2 * BH]
                            ohcur = prev[h][1][:, (tc - 1) * BH: tc * BH]
                        else:
                            gprev = gob[:, (s - 1) * 2 * BH: s * 2 * BH]
                            ohcur = ohc[:, (s - 1) * BH: s * BH]
                        g1prev = gprev[:, 0:BH]
                        g0prev = gprev[:, BH:2 * BH]

                        ps = psp.tile([KP, 2 * BH], f32, tag="ps")
                        nc.tensor.matmul(
                            ps[:, 0:BH], ewp_t[:, H0:MP], ohcur,
                            start=True, stop=False,
                        )
                        nc.tensor.matmul(
                            ps[:, BH:2 * BH], ewp_t[:, 0:H0], ohcur,
                            start=False, stop=False, skip_group_check=True,
                        )
                        nc.tensor.matmul(
                            ps[:, 0:BH], wh1_t[:, H0:MP], g1prev,
                            start=False, stop=False, skip_group_check=True,
                        )
                        nc.tensor.matmul(
                            ps[:, 0:BH], wh0_t[:, H0:MP], g0prev,
                            start=False, stop=True, skip_group_check=True,
                        )
                        nc.tensor.matmul(
                            ps[:, BH:2 * BH], wh1_t[:, 0:H0], g1prev,
                            start=False, stop=False, skip_group_check=True,
                        )
                        nc.tensor.matmul(
                            ps[:, BH:2 * BH], wh0_t[:, 0:H0], g0prev,
                            start=False, stop=True, skip_group_check=True,
                        )
                        nc.scalar.activation(
                            gob[:, s * 2 * BH:(s + 1) * 2 * BH], ps[:], AF.Tanh
                        )

                    if c >= 1 and s % lg_every == 0:
                        idx = s // lg_every
                        if idx < n_lg:
                            hh = idx // BH
                            bl = idx % BH
                            emit_logits(c - 1, hh, bl, prev[hh][0])
                prev = bufs

            for h in range(nhalf):
                pg = prev[h][0]
                last = (tc - 1) * 2 * BH
                psh = pslgp.tile([BH, HIDDEN], bf16, tag="pslg")
                nc.tensor.transpose(
                    psh[:, 0:H0], pg[:, last + BH: last + 2 * BH], ident_t[:]
                )
                nc.tensor.transpose(
                    psh[:, H0:HIDDEN],
                    pg[0:H1, last: last + BH],
                    ident_t[0:H1, 0:H1],
                )
                hb = tmpp.tile([BH, HIDDEN], f32, tag=f"hb{h}")
                nc.vector.tensor_copy(hb[:], psh[:])
                nc.gpsimd.dma_start(hout_d[h * BH:(h + 1) * BH, :], hb[:])

            for idx in range(nhalf * BH):
                hh = idx // BH
                bl = idx % BH
                emit_logits(nchunk - 1, hh, bl, prev[hh][0])


    nc.compile()
    return nc


def get_program(T=SEQ, nhalf=1, tc=128):
    key = (T, nhalf, tc)
    if key not in _PROG_CACHE:
        _PROG_CACHE[key] = _build_program(T, nhalf, tc)
    return _PROG_CACHE[key]


def _prep_inputs(x, hidden, embedding, W_e, W_h, W_o, T, nhalf):
    bf16 = ml_dtypes.bfloat16
    BH = BPC // nhalf
    x = np.asarray(x)
    hidden = np.asarray(hidden, dtype=np.float32)
    embedding = np.asarray(embedding, dtype=np.float32)
    W_e = np.asarray(W_e, dtype=np.float32)
    W_h = np.asarray(W_h, dtype=np.float32)
    W_o = np.asarray(W_o, dtype=np.float32)

    wh0 = np.zeros((KP, MP), np.float32)
    wh0[0:H0, 128:128 + H1] = W_h[0:H0, H0:HIDDEN]
    wh0[0:H0, 0:H0] = W_h[0:H0, 0:H0]
    wh1 = np.zeros((KP, MP), np.float32)
    wh1[0:H1, 128:128 + H1] = W_h[H0:HIDDEN, H0:HIDDEN]
    wh1[0:H1, 0:H0] = W_h[H0:HIDDEN, 0:H0]
    wo1 = np.zeros((KP, VOCAB), np.float32)
    wo1[0:H1, :] = W_o[H0:HIDDEN, :]
    shared = {
        "wh0": wh0.astype(bf16),
        "wh1": wh1.astype(bf16),
        "wo0": W_o[0:H0, :].astype(bf16),
        "wo1": wo1.astype(bf16),
        "embT": np.ascontiguousarray(embedding.T).astype(bf16),
        "we": W_e.astype(bf16),
    }
    in_maps = []
    for core in range(NCORES):
        m = dict(shared)
        xc = x[core * BPC:(core + 1) * BPC, :]
        hc = hidden[core * BPC:(core + 1) * BPC, :]
        for h in range(nhalf):
            xh = np.asarray(xc[h * BH:(h + 1) * BH, :T], dtype=np.int64)
            ohh = np.zeros((OHK, (T + 1) * BH), np.float32)
            cols = np.arange(T * BH)
            ohh[xh.T.reshape(-1), cols] = 1.0
            m[f"ohh{h}"] = ohh.astype(bf16)
            gh = np.ascontiguousarray(hc[h * BH:(h + 1) * BH, :].T)  # [200, BH]
            gi = np.zeros((KP, 2 * BH), np.float32)
            gi[0:H1, 0:BH] = gh[H0:HIDDEN, :]      # g1-side
            gi[0:H0, BH:2 * BH] = gh[0:H0, :]      # g0-side
            m[f"gi{h}"] = gi.astype(bf16)
        in_maps.append(m)
    return in_maps


def run_on_device(x, hidden, embedding, W_e, W_h, W_o, T=SEQ, nhalf=1, tc=128,
                  trace=False, **kw):
    from concourse.bass_utils import run_bass_kernel_spmd

    tc = min(tc, T)
    nc = get_program(T, nhalf, tc)
    in_maps = _prep_inputs(x, hidden, embedding, W_e, W_h, W_o, T, nhalf)
    res = run_bass_kernel_spmd(
        nc, in_maps, core_ids=list(range(NCORES)), trace=trace, **kw
    )
    logits = np.concatenate(
        [np.asarray(r["logits"], dtype=np.float32).reshape(BPC, T, VOCAB)
         for r in res.results],
        axis=0,
    )
    hout = np.concatenate(
        [np.asarray(r["hout"], dtype=np.float32) for r in res.results], axis=0
    )
    return (logits, hout), res


def kernel(x, hidden, embedding, W_e, W_h, W_o):
    (logits, hout), _ = run_on_device(x, hidden, embedding, W_e, W_h, W_o)
    return logits, hout
